# revision 16
# baseline (speedup 1.0000x reference)
"""MultiHeadAttention kernel for 8x TRN2 NeuronCores.

The reference module's einsum reduces the attention tensor over BOTH the
query and key axes (attn_mass = sum_{q,k} softmax(logits)_k), and softmax
rows sum to 1, so attn_mass == Lq exactly for every (batch, head). The
whole computation therefore collapses to

    out = (Lq * (V_heads @ Wv^T + bv)).reshape(N, L, E) @ Wo^T + bo

which is a single dense GEMM after folding the (block-diagonal) per-head
V-projection into the output projection:

    out = V_flat @ W_eff + b_eff
    W_eff[h*hd+a, n] = Lq * sum_b Wv[b, a] * Wo[n, h*hd+b]      (1024 x 1024)
    b_eff[n]         = Lq * sum_{h,b} Wo[n, h*hd+b] * bv[b] + bo[n]

The device kernel is the GEMM, row-sharded across 8 cores (512 rows per
core), computed in TRANSPOSED orientation: out^T[n, m] = sum_k W[k, n]
X[m, k].  PSUM bank j holds output columns j*128..(j+1)*128 on partitions
x all 512 rows on the free dim, accumulating lhsT = W-block j against
rhs = X^T k-slabs.

v2 (this file): everything rides bf16 (inputs, weights, output — PSUM
still accumulates fp32; 2e-2 tolerance leaves ~5x margin), halving HBM
traffic, and the schedule is rebuilt around the two real bottlenecks the
fp32 trace exposed:

  * HAM clock ramp: the PE runs at ~1.2 GHz until it has been
    continuously busy ~4us, and a mid-stream DMA stall re-cools it
    (the fp32 run paid ~10us at half clock after stalling).  So: a
    bf16 junk-matmul burst starts the ramp right after the preamble
    and is sized so the first real matmul's inputs have landed by the
    time it drains — the PE never idles once started.
  * DMA supply: inputs stream over THREE queues (sync HWDGE, scalar
    HWDGE, gpsimd SWDGE), each tile ordered by its consumption
    deadline.  Banks 0 and 1 are interleaved (k-offset 2) so the
    X-slab consumption rate during the arrival phase is halved.
  * Output is bf16 too (host upcasts): banks evict through the vector
    engine (bias add fused, fp32->bf16) and drain on the sync queue,
    with the last bank split into quarters to shave the tail.
"""

import numpy as np
import ml_dtypes

import concourse.bass as bass
import concourse.bacc as bacc
import concourse.mybir as mybir
from concourse.tile import TileContext
from concourse.bass_utils import run_bass_kernel_spmd

N_CORES = 8
E = 1024            # embed dim == d_model
H, HD = 16, 64      # heads, head dim
ROWS = 4096         # N * L = 2 * 2048
RPC = ROWS // N_CORES   # rows per core = 512
P = 128             # SBUF partitions
KT = E // P         # 8 contraction slabs
JT = E // P         # 8 output-column banks

# Junk-matmul warm-up burst: keeps the PE busy (HAM ramp) from preamble
# exit until the first real operands land (~3.4us at the mid p-state).
N_JUNK_512 = 9
N_JUNK_128 = 2

# MM order = availability-greedy against the MEASURED per-DMA ready
# times of the previous run (bf16 halved descriptor sizes to 1KB and
# per-descriptor overhead dominates, so X is repacked partition-major
# and rides as 256KB pairs with 2KB descriptors; X tails land EARLY
# because every bank needs them before it can evict).  Banks 0..2 run
# k0-3 while the W half-chunks trickle in from gpsimd, bank 2 finishes
# first (its k4-7 chunk lands early on scalar), then the full-block
# banks in their arrival order; bank 5 is last.
MM_ORDER = [
    (0, 0), (0, 1), (0, 2), (0, 3), (1, 0), (1, 1), (1, 2), (1, 3),
    (2, 0), (2, 1), (2, 2), (2, 3), (2, 4), (2, 5), (2, 6), (2, 7),
    (0, 4), (0, 5), (0, 6), (0, 7), (1, 4), (1, 5), (1, 6), (1, 7),
] + [(j, k) for j in (4, 7, 3, 6, 5) for k in range(KT)]
# Bank completion order implied by MM_ORDER (evictions follow it).
EVICT_ORDER = [2, 0, 1, 4, 7, 3, 6, 5]

_NC_CACHE = {}
LAST_RESULTS = None  # BassKernelResults of the most recent device run


def _build():
    f32 = mybir.dt.float32
    bf16 = mybir.dt.bfloat16
    nc = bacc.Bacc(None, target_bir_lowering=False)
    # xs is partition-major: xs[p, k*RPC + r] = X^T[k*P + p, r], so a
    # two-slab piece is 2KB/partition contiguous (2KB DMA descriptors).
    xs = nc.declare_dram_parameter("xs", [P, KT * RPC], bf16, isOutput=False)
    wc = nc.declare_dram_parameter("wc", [JT * P, E], bf16, isOutput=False)
    bw = nc.declare_dram_parameter("bw", [P, JT], f32, isOutput=False)
    outT = nc.declare_dram_parameter("outT", [E, RPC], bf16, isOutput=True)

    with TileContext(nc) as tc:
        with (
            tc.tile_pool(name="xp", bufs=1) as xp,
            tc.tile_pool(name="wp", bufs=1) as wp,
            tc.tile_pool(name="bp", bufs=1) as bp,
            tc.tile_pool(name="pp", bufs=1, space="PSUM") as pp,
            tc.tile_pool(name="op", bufs=1) as op,
        ):
            # Junk tile for the warm-up burst: memset needs no DMA and runs
            # first on gpsimd, so the PE can start right after the preamble
            # (a vector-side memset was measured to delay the burst ~1us).
            wm = bp.tile([P, RPC], bf16, name="wm", tag="wm")
            nc.gpsimd.memset(wm[:], 1.0)
            bias = bp.tile([P, JT], f32, name="bias", tag="bias")

            # X slab pairs: tile p holds slabs 2p | 2p+1 side by side.
            xp_t = [
                xp.tile([P, 2 * RPC], bf16, name=f"x{2 * k}{2 * k + 1}",
                        tag=f"x{2 * k}{2 * k + 1}")
                for k in range(KT // 2)
            ]

            def xslab(k):
                return xp_t[k // 2][:, (k % 2) * RPC:(k % 2 + 1) * RPC]

            # W chunk tiles; wmap[(j, k)] = (tile, col offset).
            wmap = {}

            def wchunk(j, k0, k1, engine):
                t = wp.tile([P, (k1 - k0) * P], bf16, name=f"w{j}_{k0}{k1}",
                            tag=f"w{j}_{k0}{k1}")
                engine.dma_start(
                    out=t[:], in_=wc[j * P:(j + 1) * P, k0 * P:k1 * P]
                )
                for k in range(k0, k1):
                    wmap[(j, k)] = (t, (k - k0) * P)

            # --- DMA schedule ------------------------------------------
            # Ordered per queue against the MEASURED arrival cadence of
            # the previous run (first piece ready ~2.5us after issue,
            # then ~1.2-2.6us per 128-256KB piece under full 8-core
            # contention).  gpsimd SWDGE keeps its measured ~1.3us chunk
            # cadence for the small early W halves; the X pairs ride the
            # HWDGE queues with 2KB descriptors; W3..W7 are full 256KB
            # blocks (2KB descriptors).
            wchunk(0, 0, 4, nc.gpsimd)
            wchunk(1, 0, 4, nc.gpsimd)
            wchunk(2, 0, 4, nc.gpsimd)
            wchunk(0, 4, 8, nc.gpsimd)
            wchunk(7, 0, 8, nc.gpsimd)
            # sync HWDGE: X pairs 01/23/45 (x01 gates the first real MM),
            # then the W1 second half and full W3/W5.
            nc.sync.dma_start(out=xp_t[0][:], in_=xs[:, 0:2 * RPC])
            nc.sync.dma_start(out=xp_t[1][:], in_=xs[:, 2 * RPC:4 * RPC])
            nc.sync.dma_start(out=xp_t[2][:], in_=xs[:, 4 * RPC:6 * RPC])
            wchunk(1, 4, 8, nc.sync)
            wchunk(3, 0, 8, nc.sync)
            wchunk(5, 0, 8, nc.sync)
            # scalar HWDGE: bias (tiny, unblocks evictions), the x67 pair
            # EARLY (every bank needs the X tail before it can finish),
            # the W2 second half, then full W4/W6.
            nc.scalar.dma_start(out=bias[:], in_=bw[:, :])
            nc.scalar.dma_start(out=xp_t[3][:], in_=xs[:, 6 * RPC:8 * RPC])
            wchunk(2, 4, 8, nc.scalar)
            wchunk(4, 0, 8, nc.scalar)
            wchunk(6, 0, 8, nc.scalar)

            ps = [
                pp.tile([P, RPC], f32, name=f"ps{j}", tag=f"ps{j}")
                for j in range(JT)
            ]

            # Warm-up burst: nonzero bf16 junk matmuls, no DMA deps.
            for i in range(N_JUNK_512):
                nc.tensor.matmul(
                    ps[i % JT], wm[:, 0:P], wm[:, :], start=True, stop=True
                )
            for i in range(N_JUNK_128):
                nc.tensor.matmul(
                    ps[(N_JUNK_512 + i) % JT][:, 0:P],
                    wm[:, 0:P], wm[:, 0:P], start=True, stop=True,
                )

            for j, k in MM_ORDER:
                t, off = wmap[(j, k)]
                nc.tensor.matmul(
                    ps[j],
                    t[:, off:off + P],
                    xslab(k),
                    start=(k == 0),
                    stop=(k == KT - 1),
                )

            # Evictions in bank-completion order: fused bias add
            # fp32->bf16 on vector, out DMAs routed to whichever queue is
            # drained when the bank completes (each engine's outs queue
            # naturally behind its remaining input FIFO traffic).  The
            # LAST bank (5) evicts in halves on vector +
            # scalar-activation with out DMAs on sync + scalar in
            # parallel, so the post-last-matmul tail is one half-bank
            # deep.
            hh = RPC // 2
            out_eng = {2: nc.gpsimd, 0: nc.gpsimd, 1: nc.gpsimd,
                       4: nc.sync, 7: nc.scalar, 3: nc.gpsimd,
                       6: nc.sync}
            for j in EVICT_ORDER[:-1]:
                o = op.tile([P, RPC], bf16, name=f"o{j}", tag=f"o{j}")
                nc.vector.tensor_scalar_add(o[:], ps[j], bias[:, j:j + 1])
                out_eng[j].dma_start(
                    out=outT[j * P:(j + 1) * P, :], in_=o[:]
                )
            o5 = op.tile([P, RPC], bf16, name="o5", tag="o5")
            nc.vector.tensor_scalar_add(o5[:, 0:hh], ps[5][:, 0:hh],
                                        bias[:, 5:6])
            nc.sync.dma_start(out=outT[5 * P:6 * P, 0:hh], in_=o5[:, 0:hh])
            nc.scalar.add(o5[:, hh:RPC], ps[5][:, hh:RPC], bias[:, 5:6])
            nc.scalar.dma_start(out=outT[5 * P:6 * P, hh:RPC],
                                in_=o5[:, hh:RPC])
    nc.compile()
    return nc


def _get_nc():
    if "bf16" not in _NC_CACHE:
        _NC_CACHE["bf16"] = _build()
    return _NC_CACHE["bf16"]


def _prep_in_maps(V, Wv, bv, Wo, bo, lq):
    V = np.ascontiguousarray(np.asarray(V, dtype=np.float32))
    Wv64 = np.asarray(Wv, np.float64)
    Wo64 = np.asarray(Wo, np.float64)
    bv64 = np.asarray(bv, np.float64)
    bo64 = np.asarray(bo, np.float64)

    # Fold per-head V-projection + output projection + attention mass (== Lq).
    Wo_r = Wo64.reshape(E, H, HD)                       # [n, h, b]
    W_eff = lq * np.einsum("ba,nhb->han", Wv64, Wo_r, optimize=True)
    W_eff = W_eff.reshape(E, E).astype(np.float32)      # [k, n]
    b_eff = (lq * np.einsum("nhb,b->n", Wo_r, bv64) + bo64).astype(np.float32)

    # wc[j*P + p, k*P + c] = W_eff[k*P + p, j*P + c]  (lhsT blocks, natural)
    wc = np.ascontiguousarray(
        W_eff.reshape(KT, P, JT, P).transpose(2, 1, 0, 3).reshape(JT * P, E)
    ).astype(ml_dtypes.bfloat16)
    bw_blk = np.ascontiguousarray(b_eff.reshape(JT, P).T)   # [p, j] fp32

    X = V.reshape(ROWS, E)
    in_maps = []
    for i in range(N_CORES):
        # Partition-major swizzle: xs_i[p, k*RPC + r] = X^T[k*P + p, r],
        # so each multi-slab DMA piece is contiguous per partition.
        xs_i = np.ascontiguousarray(
            X[i * RPC:(i + 1) * RPC, :].T.astype(ml_dtypes.bfloat16)
            .reshape(KT, P, RPC).transpose(1, 0, 2).reshape(P, KT * RPC)
        )
        in_maps.append({"xs": xs_i, "wc": wc, "bw": bw_blk})
    return in_maps


def kernel(Q, K, V, Wq, bq, Wk, bk, Wv, bv, Wo, bo, **_unused):
    global LAST_RESULTS
    n, L, e = np.asarray(V).shape
    lq = float(np.asarray(Q).shape[1])
    in_maps = _prep_in_maps(V, Wv, bv, Wo, bo, lq)
    nc = _get_nc()
    LAST_RESULTS = run_bass_kernel_spmd(nc, in_maps, list(range(N_CORES)))
    out = np.concatenate(
        [
            LAST_RESULTS.results[i]["outT"].astype(np.float32).T
            for i in range(N_CORES)
        ],
        axis=0,
    )
    return np.ascontiguousarray(out).reshape(n, L, E)
